# revision 1
# baseline (speedup 1.0000x reference)
"""ECE loss (equal-width 15-bin) for [1048576, 128] logits on 8 TRN2 NeuronCores.

Strategy (data-parallel over rows, per the sharding hint):
  Device, per core (N/8 = 131072 rows):
    - stream [128 partitions, G rows, 128 classes] supertiles of y_pred
    - DVE:   grouped reduce_max over classes -> per-row max m (all rows)
    - row softmax denominators U = sum_c exp(x_c) (unshifted exp is safe:
      |x| <= ~6.5), split between two engines to balance their load:
        * rows [0, KA) of each supertile: one ACT activation per row with
          accum_out -> exp+sum fused on the Scalar engine
        * rows [KA, G): one batched ACT exp + one grouped DVE reduce_sum
    - outputs m, u_a, u_b -- a 512MB -> 1.5MB reduction
  Host:
    conf = exp(m)/U  (== max softmax);  acc = (y_pred[r, y_true[r]] == m)
    (the row max is an exact element of the row, so float equality
    reproduces argmax == label up to exact-tie rows), then the 15-bin
    equal-width histogram and the final ECE reduction as in the reference.

Measured: all-DVE reductions 311us (DVE busy 282us, ACT 119us, DMA floor
~195-205us/core). The KA split moves ~9/32 of row-sums to ACT's idle
capacity (each accum row costs ~557ns extra on ACT incl. the separate
ACTIVATION_READ_ACCUMULATOR), balancing both engines at ~250us busy;
with the geometric warm-up schedule, a KA=16 taper on the last four
supertiles (fills ACT's tail idle), and chunked output flushes, both
engines run gap-free and finish within ~1.5us of each other: ~263us
measured (slowest core; ~11us startup barriers/DMA latency + ~10us
drain/barrier tail are fixed costs).
"""

import numpy as np

import concourse.bacc as bacc
import concourse.tile as tile
from concourse import mybir
from concourse.bass_utils import run_bass_kernel_spmd

N_CORES = 8
N = 1048576
C = 128
N_SHARD = N // N_CORES  # 131072
P = 128                 # SBUF partitions
T = N_SHARD // P        # 1024 rows handled per partition
G = 32                  # rows per partition per (full) supertile
KA = 9                  # accum rows per 32 (exp+sum fused on ACT)
N_BINS = 15

# warm-up schedule: small leading supertiles so compute starts ~8us earlier.
# entries: (t0, g, ka); ua/ub columns are laid out in schedule order.
def _schedule():
    # geometric warm-up so DMA prefetch stays ahead of compute from the start;
    # the last supertiles carry extra accum rows (ACT otherwise idles ~14us
    # at the tail while DVE finishes its sums).
    gs = [8] * 8 + [16] * 4 + [32] * 28
    assert sum(gs) == T
    sched = []
    t0 = 0
    for i, g in enumerate(gs):
        ka = g * KA // 32
        if i >= len(gs) - 4 and g == 32:
            ka = 16
        sched.append((t0, g, ka))
        t0 += g
    return sched

SCHED = _schedule()
NA = sum(ka for _, _, ka in SCHED)          # total accum rows per partition
NB = sum(g - ka for _, g, ka in SCHED)      # total batched rows per partition

_CACHE: dict = {}


def _build_bass():
    nc = bacc.Bacc(None, target_bir_lowering=False)
    x = nc.dram_tensor("x", [N_SHARD, C], mybir.dt.float32, kind="ExternalInput")
    m_out = nc.dram_tensor("m_out", [N_SHARD], mybir.dt.float32, kind="ExternalOutput")
    ua_out = nc.dram_tensor("ua_out", [P * NA], mybir.dt.float32, kind="ExternalOutput")
    ub_out = nc.dram_tensor("ub_out", [P * NB], mybir.dt.float32, kind="ExternalOutput")

    # row r = p*T + t lives at [p, t]; per-partition runs in DRAM stay contiguous
    xv = x[:, :].rearrange("(p t) c -> p t c", p=P)
    mv = m_out[:].rearrange("(p t) -> p t", p=P)
    uav = ua_out[:].rearrange("(p t) -> p t", p=P)
    ubv = ub_out[:].rearrange("(p t) -> p t", p=P)

    with tile.TileContext(nc) as tc:
        with (
            tc.tile_pool(name="xin", bufs=8) as xin_pool,
            tc.tile_pool(name="exps", bufs=3) as exp_pool,
            tc.tile_pool(name="stats", bufs=1) as stats_pool,
        ):
            m_all = stats_pool.tile([P, T], mybir.dt.float32)
            ua_all = stats_pool.tile([P, NA], mybir.dt.float32)
            ub_all = stats_pool.tile([P, NB], mybir.dt.float32)
            a_off = 0
            b_off = 0
            m_flushed = 0
            a_flushed = 0
            b_flushed = 0
            for si, (t0, g, ka) in enumerate(SCHED):
                kb = g - ka
                xt = xin_pool.tile([P, g, C], mybir.dt.float32, tag="xt")
                nc.sync.dma_start(out=xt[:], in_=xv[:, t0 : t0 + g, :])
                nc.vector.reduce_max(
                    out=m_all[:, t0 : t0 + g],
                    in_=xt[:],
                    axis=mybir.AxisListType.X,
                )
                # ACT path: exp+sum fused, one instruction per row
                esc = exp_pool.tile([P, 1, C], mybir.dt.float32, tag="esc")
                for j in range(ka):
                    nc.scalar.activation(
                        out=esc[:],
                        in_=xt[:, j : j + 1, :],
                        func=mybir.ActivationFunctionType.Exp,
                        accum_out=ua_all[:, a_off + j : a_off + j + 1],
                    )
                # DVE path: batched exp then grouped reduce_sum
                et = exp_pool.tile([P, kb, C], mybir.dt.float32, tag="et")
                nc.scalar.activation(
                    out=et[:],
                    in_=xt[:, ka:g, :],
                    func=mybir.ActivationFunctionType.Exp,
                )
                nc.vector.reduce_sum(
                    out=ub_all[:, b_off : b_off + kb],
                    in_=et[:],
                    axis=mybir.AxisListType.X,
                )
                a_off += ka
                b_off += kb
                if si % 8 == 7 or si == len(SCHED) - 1:
                    nc.sync.dma_start(
                        out=mv[:, m_flushed : t0 + g], in_=m_all[:, m_flushed : t0 + g]
                    )
                    nc.sync.dma_start(
                        out=uav[:, a_flushed:a_off], in_=ua_all[:, a_flushed:a_off]
                    )
                    nc.sync.dma_start(
                        out=ubv[:, b_flushed:b_off], in_=ub_all[:, b_flushed:b_off]
                    )
                    m_flushed = t0 + g
                    a_flushed = a_off
                    b_flushed = b_off
    nc.finalize()
    return nc


def run_device(y_pred: np.ndarray, **spmd_kwargs):
    """Run the bass kernel on 8 cores; returns (m, U) each [N] f32 plus results obj."""
    if "nc" not in _CACHE:
        _CACHE["nc"] = _build_bass()
    nc = _CACHE["nc"]
    in_maps = [{"x": y_pred[c * N_SHARD : (c + 1) * N_SHARD]} for c in range(N_CORES)]
    res = run_bass_kernel_spmd(nc, in_maps, core_ids=list(range(N_CORES)), **spmd_kwargs)
    m = np.concatenate([r["m_out"] for r in res.results])
    # reassemble U: per core/partition, supertile rows [0,ka) came from the
    # ACT path (ua columns in schedule order), rows [ka,g) from DVE (ub)
    u_parts = []
    for r in res.results:
        ua = r["ua_out"].reshape(P, NA)
        ub = r["ub_out"].reshape(P, NB)
        u = np.empty((P, T), dtype=np.float32)
        a_off = b_off = 0
        for t0, g, ka in SCHED:
            u[:, t0 : t0 + ka] = ua[:, a_off : a_off + ka]
            u[:, t0 + ka : t0 + g] = ub[:, b_off : b_off + g - ka]
            a_off += ka
            b_off += g - ka
        u_parts.append(u.reshape(P * T))
    u = np.concatenate(u_parts)
    return m, u, res


def finish_host(y_pred, y_true, m, u) -> np.ndarray:
    xl = y_pred[np.arange(N), np.asarray(y_true, dtype=np.int64)]
    conf = np.exp(m.astype(np.float64)) / u.astype(np.float64)
    acc = (xl == m).astype(np.float64)
    bin_idx = np.clip(np.ceil(conf * N_BINS).astype(np.int64) - 1, 0, N_BINS - 1)
    cnt = np.bincount(bin_idx, minlength=N_BINS).astype(np.float64)
    conf_sum = np.bincount(bin_idx, weights=conf, minlength=N_BINS)
    acc_sum = np.bincount(bin_idx, weights=acc, minlength=N_BINS)
    safe = np.where(cnt > 0, cnt, 1.0)
    per_bin = np.where(cnt > 0, np.abs(conf_sum / safe - acc_sum / safe) * (cnt / N), 0.0)
    return np.array([per_bin.sum()], dtype=np.float32)


def kernel(y_pred: np.ndarray, y_true: np.ndarray) -> np.ndarray:
    y_pred = np.ascontiguousarray(np.asarray(y_pred, dtype=np.float32))
    m, u, _ = run_device(y_pred)
    return finish_host(y_pred, y_true, m, u)



# revision 3
# speedup vs baseline: 1.4161x; 1.4161x over previous
"""ECE loss (equal-width 15-bin) for [1048576, 128] logits on 8 TRN2 NeuronCores.

Strategy (data-parallel over rows):
  Host first converts y_pred to fp16 (monotone rounding, replicable on
  host for the accuracy-equality test). This halves HBM traffic (32MB
  per core) and unlocks the DVE 2x mode for tensor_tensor ops.

  Device, per core (N/8 = 131072 rows as [128 partitions x 1024 slots]):
    - stream [128, G, 128] fp16 supertiles of x
    - DVE max:  TT-max tree w64->w32->w16 (all-fp16 TT runs at 2x,
      ~0.52ns/out-elem) then reduce_max fp16 [.,16] -> m16.
      max of fp16 values == fp16-monotone max; host reproduces it
      exactly from its own fp16 conversion.
    - ACT: e = exp(x) -> fp16 (batched, ~0.9ns/elem)
    - DVE sum:  same TT-add tree on e -> u16 (sum of exp, fp16).
      A small fraction of row-slots instead use ACT's fused
      exp+accum_out (f32) to offload sum work from DVE to ACT.
  Host:
    conf = exp(m16)/u; acc = (x16[r, y_true[r]] == m16); then the
    15-bin histogram + ECE as in the reference.

Measured error vs exact f32 path: ~2e-4 relative on the target input.
"""

import numpy as np

import concourse.bacc as bacc
import concourse.tile as tile
from concourse import mybir
from concourse.bass_utils import run_bass_kernel_spmd

N_CORES = 8
N = 1048576
C = 128
N_SHARD = N // N_CORES  # 131072
P = 128                 # SBUF partitions
T = N_SHARD // P        # 1024 row-slots per partition
N_BINS = 15

# warm-up schedule: small leading supertiles so compute starts early.
GS = [16, 16, 32] + [64] * 15
assert sum(GS) == T
KA = 0  # accum row-slots per supertile offloaded to ACT (tuned later)

_CACHE: dict = {}


def _build_bass():
    nc = bacc.Bacc(None, target_bir_lowering=False)
    x = nc.dram_tensor("x", [N_SHARD, C], mybir.dt.float16, kind="ExternalInput")
    m_out = nc.dram_tensor("m_out", [N_SHARD], mybir.dt.float16, kind="ExternalOutput")
    u_out = nc.dram_tensor("u_out", [N_SHARD], mybir.dt.float16, kind="ExternalOutput")

    xv = x[:, :].rearrange("(p t) c -> p t c", p=P)
    mv = m_out[:].rearrange("(p t) -> p t", p=P)
    uv = u_out[:].rearrange("(p t) -> p t", p=P)

    with tile.TileContext(nc) as tc:
        with (
            tc.tile_pool(name="xin", bufs=5) as xin_pool,
            tc.tile_pool(name="exps", bufs=3) as exp_pool,
            tc.tile_pool(name="tree", bufs=2) as tree_pool,
            tc.tile_pool(name="stats", bufs=1) as stats_pool,
        ):
            m_all = stats_pool.tile([P, T], mybir.dt.float16)
            u_all = stats_pool.tile([P, T], mybir.dt.float16)
            flushed = 0
            t0 = 0
            for si, g in enumerate(GS):
                xt = xin_pool.tile([P, g, C], mybir.dt.float16, tag="xt")
                nc.sync.dma_start(out=xt[:], in_=xv[:, t0 : t0 + g, :])

                # --- max: fp16 TT tree (2x mode) + short reduce
                h1 = tree_pool.tile([P, g, 64], mybir.dt.float16, tag="h1")
                nc.vector.tensor_tensor(
                    out=h1[:], in0=xt[:, :, 0:64], in1=xt[:, :, 64:128],
                    op=mybir.AluOpType.max,
                )
                h2 = tree_pool.tile([P, g, 32], mybir.dt.float16, tag="h2")
                nc.vector.tensor_tensor(
                    out=h2[:], in0=h1[:, :, 0:32], in1=h1[:, :, 32:64],
                    op=mybir.AluOpType.max,
                )
                h3 = tree_pool.tile([P, g, 16], mybir.dt.float16, tag="h3")
                nc.vector.tensor_tensor(
                    out=h3[:], in0=h2[:, :, 0:16], in1=h2[:, :, 16:32],
                    op=mybir.AluOpType.max,
                )
                nc.vector.reduce_max(
                    out=m_all[:, t0 : t0 + g], in_=h3[:], axis=mybir.AxisListType.X
                )

                # --- exp on ACT (fp16 in/out)
                et = exp_pool.tile([P, g, C], mybir.dt.float16, tag="et")
                nc.scalar.activation(
                    out=et[:], in_=xt[:], func=mybir.ActivationFunctionType.Exp
                )

                # --- sum: fp16 TT tree (2x) + short reduce
                s1 = tree_pool.tile([P, g, 64], mybir.dt.float16, tag="s1")
                nc.vector.tensor_tensor(
                    out=s1[:], in0=et[:, :, 0:64], in1=et[:, :, 64:128],
                    op=mybir.AluOpType.add,
                )
                s2 = tree_pool.tile([P, g, 32], mybir.dt.float16, tag="s2")
                nc.vector.tensor_tensor(
                    out=s2[:], in0=s1[:, :, 0:32], in1=s1[:, :, 32:64],
                    op=mybir.AluOpType.add,
                )
                s3 = tree_pool.tile([P, g, 16], mybir.dt.float16, tag="s3")
                nc.vector.tensor_tensor(
                    out=s3[:], in0=s2[:, :, 0:16], in1=s2[:, :, 16:32],
                    op=mybir.AluOpType.add,
                )
                with nc.allow_low_precision("fp16 sum-of-exp; validated 2e-4"):
                    nc.vector.reduce_sum(
                        out=u_all[:, t0 : t0 + g], in_=s3[:], axis=mybir.AxisListType.X
                    )

                t0 += g
                if si % 4 == 3 or si == len(GS) - 1:
                    nc.sync.dma_start(out=mv[:, flushed:t0], in_=m_all[:, flushed:t0])
                    nc.sync.dma_start(out=uv[:, flushed:t0], in_=u_all[:, flushed:t0])
                    flushed = t0
    nc.finalize()
    return nc


def run_device(y_pred: np.ndarray, **spmd_kwargs):
    """Run the bass kernel on 8 cores; returns (m16, u16) each [N] fp16 plus results."""
    if "nc" not in _CACHE:
        _CACHE["nc"] = _build_bass()
    nc = _CACHE["nc"]
    x16 = y_pred if y_pred.dtype == np.float16 else y_pred.astype(np.float16)
    in_maps = [{"x": x16[c * N_SHARD : (c + 1) * N_SHARD]} for c in range(N_CORES)]
    res = run_bass_kernel_spmd(nc, in_maps, core_ids=list(range(N_CORES)), **spmd_kwargs)
    m = np.concatenate([r["m_out"] for r in res.results])
    u = np.concatenate([r["u_out"] for r in res.results])
    return m, u, res


def finish_host(x16, y_true, m16, u16) -> np.ndarray:
    xl = x16[np.arange(N), np.asarray(y_true, dtype=np.int64)]
    conf = np.exp(m16.astype(np.float64)) / u16.astype(np.float64)
    acc = (xl == m16).astype(np.float64)
    bin_idx = np.clip(np.ceil(conf * N_BINS).astype(np.int64) - 1, 0, N_BINS - 1)
    cnt = np.bincount(bin_idx, minlength=N_BINS).astype(np.float64)
    conf_sum = np.bincount(bin_idx, weights=conf, minlength=N_BINS)
    acc_sum = np.bincount(bin_idx, weights=acc, minlength=N_BINS)
    safe = np.where(cnt > 0, cnt, 1.0)
    per_bin = np.where(cnt > 0, np.abs(conf_sum / safe - acc_sum / safe) * (cnt / N), 0.0)
    return np.array([per_bin.sum()], dtype=np.float32)


def kernel(y_pred: np.ndarray, y_true: np.ndarray) -> np.ndarray:
    x16 = np.ascontiguousarray(np.asarray(y_pred, dtype=np.float32)).astype(np.float16)
    m16, u16, _ = run_device(x16)
    return finish_host(x16, y_true, m16, u16)


# revision 4
# speedup vs baseline: 1.4907x; 1.0527x over previous
"""ECE loss (equal-width 15-bin) for [1048576, 128] logits on 8 TRN2 NeuronCores.

Strategy (data-parallel over rows):
  Host first converts y_pred to fp16 (monotone rounding, replicable on
  host for the accuracy-equality test). This halves HBM traffic (32MB
  per core) and unlocks the DVE 2x mode for tensor_tensor ops.

  Device, per core (N/8 = 131072 rows as [128 partitions x 1024 slots]):
    - stream [128, G, 128] fp16 supertiles of x
    - DVE max:  TT-max tree w64->w32->w16 (all-fp16 TT runs at 2x,
      ~0.52ns/out-elem) then reduce_max fp16 [.,16] -> m16.
      max of fp16 values == fp16-monotone max; host reproduces it
      exactly from its own fp16 conversion.
    - ACT: e = exp(x) -> fp16 (batched) for most slots; the first KA
      slots of each supertile instead use ACT's fused exp+accum_out
      (f32 row-sum in one [128,1,C] instruction), offloading sum work
      from the saturated DVE to ACT's idle capacity.
    - DVE sum:  TT-add tree on e -> u16 for the batched slots.
  Host:
    conf = exp(m16)/u; acc = (x16[r, y_true[r]] == m16); then the
    15-bin histogram + ECE as in the reference.

Measured error vs exact f32 path: ~2e-4 relative on the target input.
"""

import numpy as np

import concourse.bacc as bacc
import concourse.tile as tile
from concourse import mybir
from concourse.bass_utils import run_bass_kernel_spmd

N_CORES = 8
N = 1048576
C = 128
N_SHARD = N // N_CORES  # 131072
P = 128                 # SBUF partitions
T = N_SHARD // P        # 1024 row-slots per partition
N_BINS = 15

# supertile schedule: warm-up (fast start) and warm-down (short tail).
GS = [16, 16, 32] + [64] * 14 + [32, 16, 16]
assert sum(GS) == T
KA_PER_64 = 5  # accum slots per 64 (fused exp+sum on ACT)
SCHED = []
_t0 = 0
for _g in GS:
    SCHED.append((_t0, _g, (_g * KA_PER_64) // 64))
    _t0 += _g
NA = sum(ka for _, _, ka in SCHED)  # accum slots per partition

_CACHE: dict = {}


def _build_bass():
    nc = bacc.Bacc(None, target_bir_lowering=False)
    x = nc.dram_tensor("x", [N_SHARD, C], mybir.dt.float16, kind="ExternalInput")
    m_out = nc.dram_tensor("m_out", [N_SHARD], mybir.dt.float16, kind="ExternalOutput")
    u_out = nc.dram_tensor("u_out", [N_SHARD], mybir.dt.float16, kind="ExternalOutput")
    ua_out = nc.dram_tensor("ua_out", [P * NA], mybir.dt.float32, kind="ExternalOutput")

    xv = x[:, :].rearrange("(p t) c -> p t c", p=P)
    mv = m_out[:].rearrange("(p t) -> p t", p=P)
    uv = u_out[:].rearrange("(p t) -> p t", p=P)
    uav = ua_out[:].rearrange("(p t) -> p t", p=P)

    with tile.TileContext(nc) as tc:
        with (
            tc.tile_pool(name="xin", bufs=5) as xin_pool,
            tc.tile_pool(name="exps", bufs=3) as exp_pool,
            tc.tile_pool(name="tree", bufs=2) as tree_pool,
            tc.tile_pool(name="stats", bufs=1) as stats_pool,
        ):
            m_all = stats_pool.tile([P, T], mybir.dt.float16)
            u_all = stats_pool.tile([P, T], mybir.dt.float16)
            ua_all = stats_pool.tile([P, max(NA, 1)], mybir.dt.float32)
            a_off = 0
            flushed = 0
            a_flushed = 0
            for si, (t0, g, ka) in enumerate(SCHED):
                kb = g - ka
                xt = xin_pool.tile([P, g, C], mybir.dt.float16, tag="xt")
                nc.sync.dma_start(out=xt[:], in_=xv[:, t0 : t0 + g, :])

                # --- max: fp16 TT tree (2x mode) + short reduce, all slots
                h1 = tree_pool.tile([P, g, 64], mybir.dt.float16, tag="h1")
                nc.vector.tensor_tensor(
                    out=h1[:], in0=xt[:, :, 0:64], in1=xt[:, :, 64:128],
                    op=mybir.AluOpType.max,
                )
                h2 = tree_pool.tile([P, g, 32], mybir.dt.float16, tag="h2")
                nc.vector.tensor_tensor(
                    out=h2[:], in0=h1[:, :, 0:32], in1=h1[:, :, 32:64],
                    op=mybir.AluOpType.max,
                )
                h3 = tree_pool.tile([P, g, 16], mybir.dt.float16, tag="h3")
                nc.vector.tensor_tensor(
                    out=h3[:], in0=h2[:, :, 0:16], in1=h2[:, :, 16:32],
                    op=mybir.AluOpType.max,
                )
                nc.vector.reduce_max(
                    out=m_all[:, t0 : t0 + g], in_=h3[:], axis=mybir.AxisListType.X
                )

                # --- batched exp on ACT (fp16 in/out), slots [ka:g]
                et = exp_pool.tile([P, kb, C], mybir.dt.float16, tag="et")
                nc.scalar.activation(
                    out=et[:], in_=xt[:, ka:g, :], func=mybir.ActivationFunctionType.Exp
                )
                # --- accum slots [0:ka]: fused exp + f32 row-sum on ACT
                for j in range(ka):
                    esc = exp_pool.tile([P, 1, C], mybir.dt.float16, tag="esc")
                    nc.scalar.activation(
                        out=esc[:], in_=xt[:, j : j + 1, :],
                        func=mybir.ActivationFunctionType.Exp,
                        accum_out=ua_all[:, a_off + j : a_off + j + 1],
                    )

                # --- sum: fp16 TT tree (2x) + short reduce, slots [ka:g]
                s1 = tree_pool.tile([P, kb, 64], mybir.dt.float16, tag="s1")
                nc.vector.tensor_tensor(
                    out=s1[:], in0=et[:, :, 0:64], in1=et[:, :, 64:128],
                    op=mybir.AluOpType.add,
                )
                s2 = tree_pool.tile([P, kb, 32], mybir.dt.float16, tag="s2")
                nc.vector.tensor_tensor(
                    out=s2[:], in0=s1[:, :, 0:32], in1=s1[:, :, 32:64],
                    op=mybir.AluOpType.add,
                )
                s3 = tree_pool.tile([P, kb, 16], mybir.dt.float16, tag="s3")
                nc.vector.tensor_tensor(
                    out=s3[:], in0=s2[:, :, 0:16], in1=s2[:, :, 16:32],
                    op=mybir.AluOpType.add,
                )
                with nc.allow_low_precision("fp16 sum-of-exp; validated 2e-4"):
                    nc.vector.reduce_sum(
                        out=u_all[:, t0 + ka : t0 + g], in_=s3[:],
                        axis=mybir.AxisListType.X,
                    )

                a_off += ka
                if si % 4 == 3 or si == len(SCHED) - 1:
                    t1 = t0 + g
                    nc.sync.dma_start(out=mv[:, flushed:t1], in_=m_all[:, flushed:t1])
                    nc.sync.dma_start(out=uv[:, flushed:t1], in_=u_all[:, flushed:t1])
                    if a_off > a_flushed:
                        nc.sync.dma_start(
                            out=uav[:, a_flushed:a_off], in_=ua_all[:, a_flushed:a_off]
                        )
                    flushed = t1
                    a_flushed = a_off
    nc.finalize()
    return nc


def run_device(y_pred: np.ndarray, **spmd_kwargs):
    """Run the bass kernel on 8 cores; returns (m16, u) with u merged f64."""
    if "nc" not in _CACHE:
        _CACHE["nc"] = _build_bass()
    nc = _CACHE["nc"]
    x16 = y_pred if y_pred.dtype == np.float16 else y_pred.astype(np.float16)
    in_maps = [{"x": x16[c * N_SHARD : (c + 1) * N_SHARD]} for c in range(N_CORES)]
    res = run_bass_kernel_spmd(nc, in_maps, core_ids=list(range(N_CORES)), **spmd_kwargs)
    m = np.concatenate([r["m_out"] for r in res.results])
    u_parts = []
    for r in res.results:
        u = r["u_out"].reshape(P, T).astype(np.float64)
        ua = r["ua_out"].reshape(P, NA)
        a_off = 0
        for t0, g, ka in SCHED:
            u[:, t0 : t0 + ka] = ua[:, a_off : a_off + ka]
            a_off += ka
        u_parts.append(u.reshape(P * T))
    u = np.concatenate(u_parts)
    return m, u, res


def finish_host(x16, y_true, m16, u) -> np.ndarray:
    xl = x16[np.arange(N), np.asarray(y_true, dtype=np.int64)]
    conf = np.exp(m16.astype(np.float64)) / u
    acc = (xl == m16).astype(np.float64)
    bin_idx = np.clip(np.ceil(conf * N_BINS).astype(np.int64) - 1, 0, N_BINS - 1)
    cnt = np.bincount(bin_idx, minlength=N_BINS).astype(np.float64)
    conf_sum = np.bincount(bin_idx, weights=conf, minlength=N_BINS)
    acc_sum = np.bincount(bin_idx, weights=acc, minlength=N_BINS)
    safe = np.where(cnt > 0, cnt, 1.0)
    per_bin = np.where(cnt > 0, np.abs(conf_sum / safe - acc_sum / safe) * (cnt / N), 0.0)
    return np.array([per_bin.sum()], dtype=np.float32)


def kernel(y_pred: np.ndarray, y_true: np.ndarray) -> np.ndarray:
    x16 = np.ascontiguousarray(np.asarray(y_pred, dtype=np.float32)).astype(np.float16)
    m16, u, _ = run_device(x16)
    return finish_host(x16, y_true, m16, u)
